# revision 25
# baseline (speedup 1.0000x reference)
"""Trainium2 Bass kernel for nn_Custom_Final_Pooling_2D (segment_reduce).

Computes out = einsum("rn,bn->br", T, x*x) where T is the fixed binary
2x2-pooling selector built by the reference's build_pooling_matrix(32, 16):
  - T has shape [496, 1024]; only rows r0(l)+c are nonzero, where
    r0(l) = 31*l - l*(l+1)//2 + 15, for l, c in [0, 16).
  - Row r0(l)+c sums x[.., i*32+j]^2 over the 2x2 window
    i in {2l, 2l+1}, j in {2c, 2c+1}.

So the kernel is: square (ScalarE, in place), pairwise add along j
(VectorE, stride-2), then 16 pairwise adds along i written directly into
a pre-zeroed 496-wide output tile at offsets r0(l), then a contiguous
full-width DMA store (strided partial-width stores measured ~1.9x slower
per byte than contiguous full rows, so full-width wins in absolute time).

Data-parallel over 8 NeuronCores: batch dim sharded 65536 -> 8 x 8192.
"""

import numpy as np

import concourse.bacc as bacc
import concourse.mybir as mybir
from concourse.tile import TileContext
from concourse.bass_utils import run_bass_kernel_spmd

N_CORES = 8
BATCH = 65536
IMG = 32          # input image side
OUT_SIDE = 16     # pooled side
N_FEAT = IMG * IMG          # 1024
N_OUT = (2 * OUT_SIDE) * (2 * OUT_SIDE - 1) // 2  # 496
ROWS_PER_CORE = BATCH // N_CORES  # 8192

P = 128           # SBUF partitions
R = 8             # batch rows per partition per supertile
SUPER = P * R     # 1024 batch rows per supertile
N_TILES = ROWS_PER_CORE // SUPER  # 8

# Nonzero-row offsets of T: line l's 16 outputs live at columns
# r0(l) .. r0(l)+15 of the 496-wide output. Cols < 15 and >= 376 are
# always zero (as are the interior gaps); they stay at the memset value.
R0 = [31 * l - l * (l + 1) // 2 + 15 for l in range(OUT_SIDE)]

_CACHE = {}


def build_program(rows: int = ROWS_PER_CORE, r: int = R, repeat: int = 1,
                  internal_io: bool = False, mode: str = "full"):
    """Build the per-core Bass program: x [rows, 1024] -> out [rows, 496].

    repeat > 1 wraps the whole body in a hardware For_i loop that redoes
    the identical pass `repeat` times — used only for benchmarking (the
    slope over `repeat` isolates on-device time from host overhead).

    internal_io=True replaces the I/O tensors with internal DRAM buffers
    (plus a dummy [1,1] external output) so benchmark calls skip the
    256 MiB host<->device transfer entirely. The instruction stream is
    identical to the real program.

    mode: "full" (real kernel) | "in_only" | "out_only" (DMA stream
    probes for benchmarking).
    """
    nc = bacc.Bacc("TRN2", target_bir_lowering=False, debug=False,
                   num_devices=N_CORES)
    f32 = mybir.dt.float32
    if internal_io:
        x = nc.dram_tensor("xbuf", [rows, N_FEAT], f32).ap()
        out = nc.dram_tensor("obuf", [rows, N_OUT], f32).ap()
        dummy = nc.dram_tensor("out", [1, 1], f32, kind="ExternalOutput").ap()
    else:
        x = nc.dram_tensor("x", [rows, N_FEAT], f32,
                           kind="ExternalInput").ap()
        out = nc.dram_tensor("out", [rows, N_OUT], f32,
                             kind="ExternalOutput").ap()

    n_tiles = rows // (P * r)
    assert n_tiles * P * r == rows

    # Per supertile: partition p holds r consecutive batch rows.
    xv = x.rearrange("(n p r) m -> n p (r m)", p=P, r=r)
    ov = out.rearrange("(n p r) m -> n p (r m)", p=P, r=r)

    NB = 4  # persistent output buffers (zero gaps memset once)

    with TileContext(nc) as tc:
        with tc.tile_pool(name="xin", bufs=3) as xin_pool, \
             tc.tile_pool(name="y1", bufs=1) as y1_pool, \
             tc.tile_pool(name="outp", bufs=1) as out_pool:
            out_tiles = []
            for b in range(NB):
                ot = out_pool.tile([P, r * N_OUT], f32, tag=f"out{b}")
                nc.gpsimd.memset(ot[:], 0.0)
                out_tiles.append(ot)

            if internal_io:
                # zero-fill the internal input region once so the bench
                # never squares NaN/Inf garbage, and feed the dummy output
                zt = xin_pool.tile([P, r * N_FEAT], f32, tag="xt")
                nc.gpsimd.memset(zt[:], 0.0)
                for t in range(n_tiles):
                    nc.sync.dma_start(out=xv[t], in_=zt[:])
                nc.sync.dma_start(out=dummy, in_=zt[:1, :1])

            def body():
                for t in range(n_tiles):
                    if mode == "out_only":
                        nc.scalar.dma_start(out=ov[t],
                                            in_=out_tiles[t % NB][:])
                        continue
                    xt = xin_pool.tile([P, r * N_FEAT], f32)
                    nc.sync.dma_start(out=xt[:], in_=xv[t])
                    if mode == "in_only":
                        continue

                    # square in place (elementwise, same AP — safe)
                    nc.scalar.activation(xt[:], xt[:],
                                         mybir.ActivationFunctionType.Square)

                    # pool over j: y1[p, 512r], index = 512*row + 16*i + c
                    y1 = y1_pool.tile([P, r * N_FEAT // 2], f32)
                    nc.vector.tensor_add(y1[:], xt[:, 0::2], xt[:, 1::2])

                    # pool over i, scattered into the 496-wide output
                    # layout: y1 viewed [p, row, l, two, c]
                    y1v = y1[:].rearrange("p (row l two c) -> p row l two c",
                                          row=r, l=OUT_SIDE, two=2,
                                          c=OUT_SIDE)
                    ot = out_tiles[t % NB]
                    otv = ot[:].rearrange("p (row q) -> p row q", q=N_OUT)
                    for l in range(OUT_SIDE):
                        nc.vector.tensor_add(
                            otv[:, :, R0[l]:R0[l] + OUT_SIDE],
                            y1v[:, :, l, 0, :],
                            y1v[:, :, l, 1, :],
                        )

                    # contiguous full-width store, issued from the
                    # otherwise-idle GPSIMD engine (SWDGE) so its
                    # wait-for-DVE never stalls the ACT sequencer
                    nc.gpsimd.dma_start(out=ov[t], in_=ot[:])

            if repeat == 1:
                body()
            else:
                with tc.For_i(0, repeat, 1):
                    body()

    nc.compile()
    return nc


def kernel(**inputs) -> np.ndarray:
    x = np.ascontiguousarray(inputs["input_state"], dtype=np.float32)
    assert x.shape == (BATCH, N_FEAT), x.shape

    if "nc" not in _CACHE:
        _CACHE["nc"] = build_program()
    nc = _CACHE["nc"]

    shards = [x[i * ROWS_PER_CORE:(i + 1) * ROWS_PER_CORE]
              for i in range(N_CORES)]
    in_maps = [{"x": s} for s in shards]
    res = run_bass_kernel_spmd(nc, in_maps, list(range(N_CORES)))
    return np.concatenate([res.results[i]["out"] for i in range(N_CORES)],
                          axis=0)


# revision 31
# speedup vs baseline: 1.0775x; 1.0775x over previous
"""Trainium2 Bass kernel for nn_Custom_Final_Pooling_2D (segment_reduce).

Computes out = einsum("rn,bn->br", T, x*x) where T is the fixed binary
2x2-pooling selector built by the reference's build_pooling_matrix(32, 16):
  - T has shape [496, 1024]; only rows r0(l)+c are nonzero, where
    r0(l) = 31*l - l*(l+1)//2 + 15, for l, c in [0, 16).
  - Row r0(l)+c sums x[.., i*32+j]^2 over the 2x2 window
    i in {2l, 2l+1}, j in {2c, 2c+1}.

So the kernel is: square (ScalarE, in place), pairwise add along j
(VectorE, stride-2), then 16 pairwise adds along i written directly into
a pre-zeroed 496-wide output tile at offsets r0(l), then a contiguous
full-width DMA store (strided partial-width stores measured ~1.9x slower
per byte than contiguous full rows, so full-width wins in absolute time).

Data-parallel over 8 NeuronCores: batch dim sharded 65536 -> 8 x 8192.
"""

import numpy as np

import concourse.bacc as bacc
import concourse.mybir as mybir
from concourse.tile import TileContext
from concourse.bass_utils import run_bass_kernel_spmd

N_CORES = 8
BATCH = 65536
IMG = 32          # input image side
OUT_SIDE = 16     # pooled side
N_FEAT = IMG * IMG          # 1024
N_OUT = (2 * OUT_SIDE) * (2 * OUT_SIDE - 1) // 2  # 496
ROWS_PER_CORE = BATCH // N_CORES  # 8192

P = 128           # SBUF partitions
R = 8             # batch rows per partition per supertile
SUPER = P * R     # 1024 batch rows per supertile
N_TILES = ROWS_PER_CORE // SUPER  # 8

# Nonzero-row offsets of T: line l's 16 outputs live at columns
# r0(l) .. r0(l)+15 of the 496-wide output. Cols < 15 and >= 376 are
# always zero (as are the interior gaps); they stay at the memset value.
R0 = [31 * l - l * (l + 1) // 2 + 15 for l in range(OUT_SIDE)]

# The device writes a compact [rows, OUT_W] output holding columns
# OUT_LO..OUT_HI of each 496-wide row (contiguous rows -> full write
# bandwidth, 27% fewer bytes); the host pads the always-zero head/tail
# columns during the gather step.
OUT_LO = R0[0]                        # 15
OUT_HI = R0[OUT_SIDE - 1] + OUT_SIDE  # 376
OUT_W = OUT_HI - OUT_LO               # 361

_CACHE = {}


def build_program(rows: int = ROWS_PER_CORE, r: int = R, repeat: int = 1,
                  internal_io: bool = False, mode: str = "full"):
    """Build the per-core Bass program: x [rows, 1024] -> out [rows, 496].

    repeat > 1 wraps the whole body in a hardware For_i loop that redoes
    the identical pass `repeat` times — used only for benchmarking (the
    slope over `repeat` isolates on-device time from host overhead).

    internal_io=True replaces the I/O tensors with internal DRAM buffers
    (plus a dummy [1,1] external output) so benchmark calls skip the
    256 MiB host<->device transfer entirely. The instruction stream is
    identical to the real program.

    mode: "full" (real kernel) | "in_only" | "out_only" (DMA stream
    probes for benchmarking).
    """
    nc = bacc.Bacc("TRN2", target_bir_lowering=False, debug=False,
                   num_devices=N_CORES)
    f32 = mybir.dt.float32
    if internal_io:
        x = nc.dram_tensor("xbuf", [rows, N_FEAT], f32).ap()
        out = nc.dram_tensor("obuf", [rows, OUT_W], f32).ap()
        dummy = nc.dram_tensor("out", [1, 1], f32, kind="ExternalOutput").ap()
    else:
        x = nc.dram_tensor("x", [rows, N_FEAT], f32,
                           kind="ExternalInput").ap()
        out = nc.dram_tensor("out", [rows, OUT_W], f32,
                             kind="ExternalOutput").ap()

    n_tiles = rows // (P * r)
    assert n_tiles * P * r == rows

    # Per supertile: partition p holds r consecutive batch rows.
    xv = x.rearrange("(n p r) m -> n p (r m)", p=P, r=r)
    ov = out.rearrange("(n p r) m -> n p (r m)", p=P, r=r)

    NB = 3  # persistent output buffers (zero gaps memset once)

    with TileContext(nc) as tc:
        with tc.tile_pool(name="xin", bufs=3) as xin_pool, \
             tc.tile_pool(name="y1", bufs=2) as y1_pool, \
             tc.tile_pool(name="outp", bufs=1) as out_pool:
            out_tiles = []
            for b in range(NB):
                ot = out_pool.tile([P, r * OUT_W], f32, tag=f"out{b}")
                nc.gpsimd.memset(ot[:], 0.0)
                out_tiles.append(ot)

            if internal_io:
                # zero-fill the internal input region once so the bench
                # never squares NaN/Inf garbage, and feed the dummy output
                zt = xin_pool.tile([P, r * N_FEAT], f32, tag="xt")
                nc.gpsimd.memset(zt[:], 0.0)
                for t in range(n_tiles):
                    nc.sync.dma_start(out=xv[t], in_=zt[:])
                nc.sync.dma_start(out=dummy, in_=zt[:1, :1])

            def body():
                for t in range(n_tiles):
                    if mode == "out_only":
                        nc.scalar.dma_start(out=ov[t],
                                            in_=out_tiles[t % NB][:])
                        continue
                    xt = xin_pool.tile([P, r * N_FEAT], f32)
                    nc.sync.dma_start(out=xt[:], in_=xv[t])
                    if mode == "in_only":
                        continue

                    # square in place (elementwise, same AP — safe)
                    nc.scalar.activation(xt[:], xt[:],
                                         mybir.ActivationFunctionType.Square)

                    # pool over j: y1[p, 512r], index = 512*row + 16*i + c
                    y1 = y1_pool.tile([P, r * N_FEAT // 2], f32)
                    nc.vector.tensor_add(y1[:], xt[:, 0::2], xt[:, 1::2])

                    # pool over i, scattered into the 496-wide output
                    # layout: y1 viewed [p, row, l, two, c]
                    y1v = y1[:].rearrange("p (row l two c) -> p row l two c",
                                          row=r, l=OUT_SIDE, two=2,
                                          c=OUT_SIDE)
                    ot = out_tiles[t % NB]
                    otv = ot[:].rearrange("p (row q) -> p row q", q=OUT_W)
                    for l in range(OUT_SIDE):
                        lo = R0[l] - OUT_LO
                        nc.vector.tensor_add(
                            otv[:, :, lo:lo + OUT_SIDE],
                            y1v[:, :, l, 0, :],
                            y1v[:, :, l, 1, :],
                        )

                    # contiguous full-width store, issued from the
                    # otherwise-idle GPSIMD engine (SWDGE) so its
                    # wait-for-DVE never stalls the ACT sequencer
                    nc.gpsimd.dma_start(out=ov[t], in_=ot[:])

            if repeat == 1:
                body()
            else:
                with tc.For_i(0, repeat, 1):
                    body()

    nc.compile()
    return nc


def kernel(**inputs) -> np.ndarray:
    x = np.ascontiguousarray(inputs["input_state"], dtype=np.float32)
    assert x.shape == (BATCH, N_FEAT), x.shape

    if "nc" not in _CACHE:
        _CACHE["nc"] = build_program()
    nc = _CACHE["nc"]

    shards = [x[i * ROWS_PER_CORE:(i + 1) * ROWS_PER_CORE]
              for i in range(N_CORES)]
    in_maps = [{"x": s} for s in shards]
    res = run_bass_kernel_spmd(nc, in_maps, list(range(N_CORES)))

    # gather + unshard: pad the always-zero head/tail columns host-side
    full = np.zeros((BATCH, N_OUT), dtype=np.float32)
    full[:, OUT_LO:OUT_HI] = np.concatenate(
        [res.results[i]["out"] for i in range(N_CORES)], axis=0)
    return full
